# revision 27
# baseline (speedup 1.0000x reference)
"""Causal self-attention (B=1, T=4096, C=768, H=12, hd=64) on 8 trn2 NeuronCores.

v2: bf16 everywhere on-device, rope via 128x128 permutation matmul, cheap-AV
orientation (exp-scores stationary, 65-col AV matmuls), band-padded causal
schedule, large strided exp spans on the Activation engine.

Launch 1 (sequence-parallel): core c computes q^T,k^T (rope'd, bf16) and v
  (natural layout, bf16) for rows [512c, 512c+512).
  rope(u) = u*cos + (P@u)*sin with P the rotate-half permutation, applied
  post-bias, so no rotated weight/bias copies are needed.

Launch 2 (query-block-parallel): core c owns query blocks [31-c, 16+c, 15-c, c]
  (counts 32-c, 17+c, 16-c, 1+c -- descending; padded per-slot tile counts
  32/24/16/8 -> identical SPMD program, band widths 512/384/256/128).
  Scores S^T[kv, q] via k-tile stationary matmuls with 4 mask channels;
  exp on ScalarE (scale=1/8, no row-max needed for N(0,1) logits); AV with
  exp-tile stationary and v moving (65 cols incl. ones-column denominator);
  per-slot normalization via per-partition reciprocal scalar on DVE; PE
  transposes assemble y^T for the output projection.
"""

import numpy as np
import ml_dtypes

import concourse.bass as bass
import concourse.bacc as bacc
import concourse.tile as tile
from concourse import mybir
from concourse.bass_utils import run_bass_kernel_spmd

F32 = mybir.dt.float32
BF16 = mybir.dt.bfloat16
NPBF = ml_dtypes.bfloat16

T, C, H, HD = 4096, 768, 12, 64
NCORES = 8
RPC = T // NCORES          # rows per core in launch 1 (512)
NT = T // 128              # kv tiles (32)
MASK = -2000.0             # additive; *0.125 -> exp underflows to 0
ROPE_BASE = 10000.0

# launch-2 slot structure: slot s of core c handles query block BLOCKS[c][s]
BLOCKS = [[31 - c, 16 + c, 15 - c, c] for c in range(NCORES)]
PAD = [32, 24, 16, 8]      # padded kv-tile counts per slot (max over cores)

# band width (q columns) for kv tile t: 128 * #{slots j : PAD[j] > t}
def _w(t):
    return 128 * sum(1 for p in PAD if p > t)

# score groups: list of (tiles, sub) where sub[i] = (bank_i, off, width)
# s2 psum tile is [128, 3, 512] (3 banks); each tile's scores sit in one bank.
GROUPS = []
GROUPS.append(([0, 1, 2],    [(0, 0, 512), (1, 0, 512), (2, 0, 512)]))
GROUPS.append(([3, 4, 5],    [(0, 0, 512), (1, 0, 512), (2, 0, 512)]))
GROUPS.append(([6, 7],       [(0, 0, 512), (1, 0, 512)]))      # bank 2 = diag
GROUPS.append(([8, 9, 10],   [(0, 0, 384), (1, 0, 384), (2, 0, 384)]))
GROUPS.append(([11, 12, 13], [(0, 0, 384), (1, 0, 384), (2, 0, 384)]))
GROUPS.append(([14, 15],     [(0, 0, 384), (1, 0, 384)]))
GROUPS.append(([16, 17, 18, 19, 20, 21],
               [(0, 0, 256), (0, 256, 256), (1, 0, 256), (1, 256, 256),
                (2, 0, 256), (2, 256, 256)]))
GROUPS.append(([22, 23],     [(0, 0, 256), (0, 256, 256)]))
GROUPS.append(([24, 25, 26, 27, 28, 29, 30, 31],
               [(0, 0, 128), (0, 128, 128), (0, 256, 128), (0, 384, 128),
                (1, 0, 128), (1, 128, 128), (1, 256, 128), (1, 384, 128)]))
DIAG_GROUP = 2             # diag scores live in GROUPS[2]'s bank 2


def _build_l1(reps=1):
    nc = bacc.Bacc("TRN2", target_bir_lowering=False, debug=False,
                   num_devices=NCORES)
    XT = nc.dram_tensor("xt", [C, RPC], BF16, kind="ExternalInput")
    WA = nc.dram_tensor("wa", [C, 3 * C], BF16, kind="ExternalInput")
    PT = nc.dram_tensor("pt", [128, 128], BF16, kind="ExternalInput")
    BQK = nc.dram_tensor("bqk", [128, 12], F32, kind="ExternalInput")
    COS = nc.dram_tensor("cos", [128, RPC], BF16, kind="ExternalInput")
    SIN = nc.dram_tensor("sin", [128, RPC], BF16, kind="ExternalInput")
    QKT = nc.dram_tensor("qkt", [2 * C, RPC], BF16, kind="ExternalOutput")
    VO = nc.dram_tensor("vo", [RPC, C], BF16, kind="ExternalOutput")

    with tile.TileContext(nc) as tc:
        with (
            tc.tile_pool(name="singles", bufs=1) as singles,
            tc.tile_pool(name="tmp", bufs=3) as tmp,
            tc.tile_pool(name="ps", bufs=2, space="PSUM") as ps,
        ):
            wa_r = WA.rearrange("(k p) n -> p k n", p=128)
            xt_r = XT.rearrange("(k p) n -> p k n", p=128)
            xt_t = []
            for k in range(6):
                xk = singles.tile([128, RPC], BF16, tag=f"xt{k}")
                nc.sync.dma_start(out=xk, in_=xt_r[:, k, :])
                xt_t.append(xk)
            wa_t = []
            for m in range(12):
                wam = singles.tile([128, 6, 128], BF16, tag=f"wa{m}")
                nc.sync.dma_start(out=wam,
                                  in_=wa_r[:, :, 128 * m:128 * (m + 1)])
                wa_t.append(wam)
            wv_sb = singles.tile([128, 6, C], BF16)
            for k in range(6):
                nc.sync.dma_start(out=wv_sb[:, k, :],
                                  in_=wa_r[:, k, 2 * C:3 * C])
            pt_sb = singles.tile([128, 128], BF16)
            nc.gpsimd.dma_start(out=pt_sb, in_=PT[:])
            bqk_sb = singles.tile([128, 12], F32)
            nc.gpsimd.dma_start(out=bqk_sb, in_=BQK[:])
            cos_sb = singles.tile([128, RPC], BF16)
            nc.gpsimd.dma_start(out=cos_sb, in_=COS[:])
            sin_sb = singles.tile([128, RPC], BF16)
            nc.gpsimd.dma_start(out=sin_sb, in_=SIN[:])

            def body(_=None):
                # q^T, k^T with RoPE: 12 channel tiles of 128
                for m in range(12):
                    ps_a = ps.tile([128, RPC], F32, tag="psa")
                    for k in range(6):
                        nc.tensor.matmul(
                            ps_a, wa_t[m][:, k, :],
                            xt_t[k], start=(k == 0), stop=(k == 5))
                    a_sb = tmp.tile([128, RPC], BF16, tag="a")
                    nc.scalar.activation(a_sb, ps_a,
                                         mybir.ActivationFunctionType.Identity,
                                         bias=bqk_sb[:, m:m + 1])
                    ps_b = ps.tile([128, RPC], F32, tag="psb")
                    nc.tensor.matmul(ps_b, pt_sb, a_sb, start=True, stop=True)
                    t1 = tmp.tile([128, RPC], BF16, tag="t1")
                    nc.vector.tensor_mul(t1, a_sb, cos_sb)
                    t2 = tmp.tile([128, RPC], BF16, tag="t2")
                    nc.vector.tensor_mul(t2, ps_b, sin_sb)
                    o_sb = tmp.tile([128, RPC], BF16, tag="o")
                    nc.vector.tensor_add(o_sb, t1, t2)
                    nc.sync.dma_start(out=QKT[128 * m:128 * (m + 1), :], in_=o_sb)

                # v in natural layout: 4 row tiles x (512 + 256) cols
                for qt in range(4):
                    for n0, nw in ((0, 512), (512, 256)):
                        ps_v = ps.tile([128, 512], F32, tag="psv")
                        for k in range(6):
                            nc.tensor.matmul(
                                ps_v[:, :nw],
                                xt_t[k][:, 128 * qt:128 * (qt + 1)],
                                wv_sb[:, k, n0:n0 + nw],
                                start=(k == 0), stop=(k == 5))
                        vo_sb = tmp.tile([128, 512], BF16, tag="vo")
                        nc.scalar.copy(vo_sb[:, :nw], ps_v[:, :nw])
                        nc.sync.dma_start(
                            out=VO[128 * qt:128 * (qt + 1), n0:n0 + nw],
                            in_=vo_sb[:, :nw])

            if reps == 1:
                body()
            else:
                with tc.For_i(0, reps, 1):
                    body()
    nc.finalize()
    return nc


DEBUG_Y = False


def _build_l2(reps=1):
    nc = bacc.Bacc("TRN2", target_bir_lowering=False, debug=False,
                   num_devices=NCORES)
    if DEBUG_Y:
        YDBG = nc.dram_tensor("ydbg", [H, 128, 4, 128], F32,
                              kind="ExternalOutput")
    # per-head merged loads: KQ = [k^T(4096) | q^T(512) | ktd(512)] on 64
    # rows; VPK = [vh(32*65) | vd(4*65)] on 128 rows; KMQM = mask rows
    KQ = nc.dram_tensor("kq", [H, 64, 5120], BF16, kind="ExternalInput")
    KMQM = nc.dram_tensor("kmqm", [4, 4608], BF16, kind="ExternalInput")
    VPK = nc.dram_tensor("vpk", [H, 128, 36 * (HD + 1)], BF16,
                         kind="ExternalInput")
    TRI = nc.dram_tensor("tri", [128, 512], F32, kind="ExternalInput")
    WP = nc.dram_tensor("wp", [C, C], BF16, kind="ExternalInput")
    BP = nc.dram_tensor("bp", [1, C], F32, kind="ExternalInput")
    IDN = nc.dram_tensor("idn", [128, 128], BF16, kind="ExternalInput")
    OUT = nc.dram_tensor("out", [512, C], F32, kind="ExternalOutput")

    with tile.TileContext(nc) as tc:
        with (
            tc.tile_pool(name="singles", bufs=1) as singles,
            tc.tile_pool(name="kp", bufs=2) as kp,
            tc.tile_pool(name="vhp", bufs=2) as vhp,
            tc.tile_pool(name="esp", bufs=5) as esp,
            tc.tile_pool(name="tailp", bufs=2) as tailp,
            tc.tile_pool(name="sp", bufs=2, space="PSUM") as sp,
            tc.tile_pool(name="yp", bufs=1, space="PSUM") as yp,
            tc.tile_pool(name="ytp", bufs=1, space="PSUM") as ytp,
        ):
            # one-time loads (SWDGE queue, ordered by first use: qm/km
            # early columns gate head-0/1 scores; tri gates head-0 g2;
            # wp/bp only gate the proj tail)
            wp_sb = singles.tile([128, 6, C], BF16)
            bp_sb = singles.tile([128, C], F32)
            tri_sb = singles.tile([128, 512], F32)
            idn_sb = singles.tile([128, 128], BF16)
            yt_sb = singles.tile([128, 6, 512], BF16)

            # prefill mask rows of the rotating kq buffers (persist across
            # heads and reps: per-head DMAs only touch rows 0:64)
            kq_bufs = []
            for i in range(2):
                kb = kp.tile([68, 5120], BF16, tag="kq")
                kq_bufs.append(kb)
            nc.gpsimd.dma_start(out=kq_bufs[0][64:68, 0:4608], in_=KMQM[:])
            nc.gpsimd.dma_start(out=tri_sb, in_=TRI[:])
            nc.gpsimd.dma_start(out=kq_bufs[1][64:68, 0:4608], in_=KMQM[:])
            nc.gpsimd.dma_start(out=idn_sb, in_=IDN[:])

            def load_head(h):
                kq = kp.tile([68, 5120], BF16, tag="kq")
                nc.sync.dma_start(out=kq[0:64, :], in_=KQ[h])
                vpk = vhp.tile([128, 36 * (HD + 1)], BF16, tag="vpk")
                nc.gpsimd.dma_start(out=vpk, in_=VPK[h])
                return kq, vpk

            def compute_head(h, tiles, prev_tail=None, prefetch=None):
                kq, vpk = tiles
                y_ps = yp.tile([128, 4, 128], F32, tag="y")

                es = [None] * len(GROUPS)
                s2_d = [None]

                def emit_scores(g):
                    ts, sub = GROUPS[g]
                    s2 = sp.tile([128, 3, 512], F32, tag="s2")
                    for t, (bk, off, w) in zip(ts, sub):
                        nc.tensor.matmul(
                            s2[:, bk, off:off + w],
                            kq[:, 128 * t:128 * (t + 1)],
                            kq[:, 4096:4096 + w],
                            start=True, stop=True)
                    if g == DIAG_GROUP:
                        for j in range(4):
                            nc.tensor.matmul(
                                s2[:, 2, 128 * j:128 * (j + 1)],
                                kq[0:64, 4608 + 128 * j:4608 + 128 * (j + 1)],
                                kq[0:64, 4096 + 128 * j:4096 + 128 * (j + 1)],
                                start=True, stop=True)
                        nc.vector.tensor_add(s2[:, 2, :], s2[:, 2, :], tri_sb)
                        s2_d[0] = s2
                    return s2

                def emit_exp(g, s2):
                    ts, sub = GROUPS[g]
                    e = esp.tile([128, 3, 512], BF16, tag="es")
                    n = len(ts)
                    w = sub[0][2]
                    if w == 384:
                        # one bank-strided span over n banks x w cols
                        nc.scalar.activation(
                            e[:, 0:n, 0:w], s2[:, 0:n, 0:w],
                            mybir.ActivationFunctionType.Exp, scale=0.125)
                    else:
                        # tiles packed contiguously inside banks; the diag
                        # group also exps its bank-2 diag scores (post tri)
                        nb = 3 if g == DIAG_GROUP else (len(ts) * w + 511) // 512
                        nc.scalar.activation(
                            e[:, 0:nb, :], s2[:, 0:nb, :],
                            mybir.ActivationFunctionType.Exp, scale=0.125)
                    es[g] = e

                def emit_av(g):
                    ts, sub = GROUPS[g]
                    e = es[g]
                    for t, (bk, off, w) in zip(ts, sub):
                        for j in range(w // 128):
                            # start=True clears has_written for the WHOLE
                            # bank, so only the very first matmul into the y
                            # bank may set it; later first-writes per region
                            # overwrite via the cleared has_written bits.
                            nc.tensor.matmul(
                                y_ps[:, j, 0:HD + 1],
                                e[:, bk, off + 128 * j:off + 128 * (j + 1)],
                                vpk[:, 65 * t:65 * t + 65],
                                start=(t == 0 and j == 0), stop=False,
                                skip_group_check=True)

                # pipelined: scores(g) ; av(g-2) ; exp(g)
                nxt = None
                for g in range(len(GROUPS)):
                    s2 = emit_scores(g)
                    if g == 1 and prefetch is not None:
                        nxt = prefetch()   # emit next head's loads after the
                                           # first scores so head h's matmuls
                                           # don't wait on h+1's DMA queue
                    if g == 2 and prev_tail is not None:
                        prev_tail()    # prev head's PE transposes, deferred
                    if g >= 2:
                        emit_av(g - 2)
                    emit_exp(g, s2)
                emit_av(len(GROUPS) - 2)
                emit_av(len(GROUPS) - 1)
                e_d = es[DIAG_GROUP]
                for j in range(4):
                    nc.tensor.matmul(
                        y_ps[:, j, 0:HD + 1],
                        e_d[:, 2, 128 * j:128 * (j + 1)],
                        vpk[:, 2080 + 65 * j:2080 + 65 * (j + 1)],
                        start=False, stop=True, skip_group_check=True)

                if DEBUG_Y:
                    ydbg_sb = tailp.tile([128, 4, 128], F32, tag="ydbg")
                    nc.vector.tensor_copy(ydbg_sb, y_ps)
                    nc.sync.dma_start(out=YDBG[h], in_=ydbg_sb)
                def tail():
                    # normalize + transpose into yt_sb (deferred into the
                    # next head, after its tri-add, so the DVE queue is not
                    # head-blocked waiting on this head's last AV)
                    rec = tailp.tile([128, 4], F32, tag="rec")
                    nc.vector.reciprocal(rec, y_ps[:, :, HD])
                    yn = tailp.tile([128, 4, HD], BF16, tag="yn")
                    for j in range(4):
                        nc.vector.tensor_scalar(
                            yn[:, j, :], y_ps[:, j, 0:HD], rec[:, j:j + 1],
                            None, mybir.AluOpType.mult)
                    yt_ps = ytp.tile([64, 4, 128], BF16, tag="yt")
                    r0 = 64 * (h % 2)
                    for j in range(4):
                        nc.tensor.transpose(yt_ps[:, j, :], yn[:, j, :], idn_sb)
                        nc.vector.tensor_copy(
                            yt_sb[r0:r0 + 64, h // 2, 128 * j:128 * (j + 1)],
                            yt_ps[:, j, :])
                return tail, nxt

            def body(_=None):
                cur = load_head(0)
                nc.gpsimd.dma_start(
                    out=wp_sb, in_=WP.rearrange("(k p) n -> p k n", p=128))
                nc.gpsimd.dma_start(out=bp_sb, in_=bass.AP(
                    tensor=BP, offset=0, ap=[[0, 128], [1, C]]))
                tail = None
                for h in range(H):
                    pf = (lambda hh=h + 1: load_head(hh)) if h + 1 < H else None
                    tail, cur = compute_head(h, cur, prev_tail=tail,
                                             prefetch=pf)
                tail()
                # output projection: OUT[q, :] = y^T.T @ WP + BP
                for qt in range(4):
                    po = sp.tile([128, 3, 512], F32, tag="s2")
                    for bi, (n0, nw) in enumerate(((0, 512), (512, 256))):
                        for k in range(6):
                            nc.tensor.matmul(
                                po[:, bi, 0:nw],
                                yt_sb[:, k, 128 * qt:128 * (qt + 1)],
                                wp_sb[:, k, n0:n0 + nw],
                                start=(k == 0), stop=(k == 5))
                    ob = tailp.tile([128, C], F32, tag="ob")
                    nc.vector.tensor_add(ob[:, 0:512], po[:, 0, :],
                                         bp_sb[:, 0:512])
                    nc.vector.tensor_add(ob[:, 512:768], po[:, 1, 0:256],
                                         bp_sb[:, 512:768])
                    nc.sync.dma_start(out=OUT[128 * qt:128 * (qt + 1), :], in_=ob)

            if reps == 1:
                body()
            else:
                with tc.For_i(0, reps, 1):
                    body()
    nc.finalize()
    return nc


_CACHE = {}


def _get(name, builder):
    if name not in _CACHE:
        _CACHE[name] = builder()
    return _CACHE[name]


def _rot_mat():
    """P such that P @ u = rotate_half(u) on the 128-channel (2-head) axis."""
    P = np.zeros((128, 128), np.float32)
    for base in (0, 64):
        for i in range(32):
            P[base + i, base + i + 32] = -1.0
            P[base + 32 + i, base + i] = 1.0
    return P


def _prep_l1_inputs(x, w_attn, b_attn):
    xT = np.ascontiguousarray(x[0].T)                       # [C, T]
    bqk = np.ascontiguousarray(b_attn[:2 * C].reshape(12, 128).T)
    PT = np.ascontiguousarray(_rot_mat().T).astype(NPBF)
    inv_freq = (1.0 / ROPE_BASE ** (np.arange(0, HD, 2, dtype=np.float64) / HD))
    d_idx = np.arange(128) % (HD // 2)
    in_maps = []
    for c in range(NCORES):
        t_rng = np.arange(RPC * c, RPC * (c + 1), dtype=np.float64)
        ang = np.outer(inv_freq[d_idx], t_rng)              # [128, RPC]
        in_maps.append({
            "xt": np.ascontiguousarray(xT[:, RPC * c:RPC * (c + 1)]).astype(NPBF),
            "wa": w_attn.astype(NPBF), "pt": PT,
            "bqk": bqk.astype(np.float32),
            "cos": np.cos(ang).astype(NPBF),
            "sin": np.sin(ang).astype(NPBF),
        })
    return in_maps


def _perm_v(v3):
    """[T', H, HD+1] -> [H, 128, (T'/128)*(HD+1)] partition-major."""
    tt = v3.shape[0]
    v4 = v3.reshape(tt // 128, 128, H, HD + 1).transpose(2, 1, 0, 3)
    return np.ascontiguousarray(v4.reshape(H, 128, (tt // 128) * (HD + 1)))


def _prep_l2_inputs(QT_all, KT_all, Vp, w_proj, bp1):
    QT_all = np.asarray(QT_all).astype(NPBF)
    KT_all = np.asarray(KT_all).astype(NPBF)
    Vp = np.asarray(Vp).astype(NPBF)
    qm = np.zeros((4, 512), NPBF)
    for s in range(4):
        qm[s, 128 * s:128 * (s + 1)] = 1.0
    tri1 = np.where(np.arange(128)[None, :] >= np.arange(128)[:, None],
                    0.0, MASK).astype(np.float32)
    tri = np.ascontiguousarray(np.tile(tri1, (1, 4)))       # [128, 512]
    idn = np.eye(128, dtype=np.float32).astype(NPBF)
    Vpp = _perm_v(Vp)
    in_maps = []
    for c in range(NCORES):
        blocks = BLOCKS[c]
        counts = [b + 1 for b in blocks]
        qt_c = np.concatenate(
            [QT_all[:, 128 * b:128 * (b + 1)] for b in blocks], axis=1)
        km = np.zeros((4, T), NPBF)
        for s in range(4):
            km[s, 128 * (counts[s] - 1):] = MASK
        ktd = np.concatenate(
            [KT_all[:, 128 * b:128 * (b + 1)] for b in blocks], axis=1)
        vd = _perm_v(np.concatenate(
            [Vp[128 * b:128 * (b + 1)] for b in blocks], axis=0))
        kq = np.empty((H, 64, 5120), NPBF)
        vpk = np.empty((H, 128, 36 * (HD + 1)), NPBF)
        for h in range(H):
            kq[h, :, 0:T] = KT_all[64 * h:64 * h + 64]
            kq[h, :, T:T + 512] = qt_c[64 * h:64 * h + 64]
            kq[h, :, T + 512:] = ktd[64 * h:64 * h + 64]
            vpk[h, :, 0:NT * (HD + 1)] = Vpp[h]
            vpk[h, :, NT * (HD + 1):] = vd[h]
        kmqm = np.concatenate([km, qm], axis=1)
        in_maps.append({
            "kq": kq, "kmqm": np.ascontiguousarray(kmqm), "vpk": vpk,
            "tri": tri,
            "wp": w_proj.astype(NPBF), "bp": bp1.reshape(1, C).astype(np.float32),
            "idn": idn,
        })
    return in_maps


def kernel(x, w_attn, b_attn, w_proj, b_proj):
    x = np.asarray(x, np.float32)
    w_attn = np.asarray(w_attn, np.float32)
    b_attn = np.asarray(b_attn, np.float32)
    w_proj = np.asarray(w_proj, np.float32)
    b_proj = np.asarray(b_proj, np.float32)

    nc1 = _get("l1", _build_l1)
    res1 = run_bass_kernel_spmd(nc1, _prep_l1_inputs(x, w_attn, b_attn),
                                list(range(NCORES))).results

    QT_all = np.concatenate([res1[c]["qkt"][:C] for c in range(NCORES)], axis=1)
    KT_all = np.concatenate([res1[c]["qkt"][C:] for c in range(NCORES)], axis=1)
    V_all = np.concatenate([res1[c]["vo"] for c in range(NCORES)], axis=0)
    Vp = np.ones((T, H, HD + 1), np.float32)
    Vp[:, :, :HD] = V_all.astype(np.float32).reshape(T, H, HD)
    bp1 = b_proj + b_attn[2 * C:] @ w_proj

    nc2 = _get("l2", _build_l2)
    res2 = run_bass_kernel_spmd(nc2, _prep_l2_inputs(QT_all, KT_all, Vp,
                                                     w_proj, bp1),
                                list(range(NCORES))).results

    out = np.empty((T, C), np.float32)
    for c in range(NCORES):
        for s, b in enumerate(BLOCKS[c]):
            out[128 * b:128 * (b + 1)] = res2[c]["out"][128 * s:128 * (s + 1)]
    return out[None]
